# revision 1
# baseline (speedup 1.0000x reference)
"""GraphTransformer (TransformerConv + mean-pool) on 8 trn2 NeuronCores.

Strategy (two launches, nodes sharded 8 ways):
  Launch A (per core, 6250 nodes + pad -> 6272):
      h = x @ W_emb + b_emb           (computed transposed, hT, via W^T @ x^T)
      qkv = h @ [Wq|Wk|Wv] + b        -> bf16 [6272, 1536] per core
      skip = h @ Wskip + bskip        -> fp32 [6272, 64]  per core
  Host: assemble full Q,K,V; sort edges by dst; group per dst-tile (128 dst
      nodes, capacity 9*128 edge slots); gather per-edge rows
      qg=Q[dst], kg=K[src], vg=V[src]; build one-hot indicator matrices.
  Launch B (per core, 49 dst tiles x 9 chunks of 128 edges):
      s[e,h]   = sum_c qg[e,hc]*kg[e,hc]        (DVE mult + grouped reduce)
      w[e,h]   = exp(s*scale)                   (ACT, broadcast-expanded)
      num[d,:] += ind_ed^T @ (w*vg)             (TensorE scatter via one-hot)
      den[d,h] += ind_ed^T @ w
      out[d,:] = mean_h(num/den) + skip[d,:]
      pooled[g,:] += ind_ng^T @ out             (TensorE, per-graph partial)
  Host: sum partial pooled over cores, divide by graph node counts.
"""

import numpy as np
import ml_dtypes

import concourse.bass as bass
from concourse import bacc
import concourse.mybir as mybir
import concourse.tile as tile
from concourse import bass_utils
from concourse.bass import ts

BF16 = mybir.dt.bfloat16
F32 = mybir.dt.float32
NP_BF16 = ml_dtypes.bfloat16

N, E, B = 50000, 400000, 64
IN_DIM, OUT_DIM, HEADS = 768, 64, 8
HC = HEADS * OUT_DIM  # 512
NCORES = 8
NPC = N // NCORES  # 6250 nodes per core
TILES = 49  # dst tiles per core (49*128 = 6272 >= 6250)
NPAD = TILES * 128  # 6272
CHUNKS = 9  # edge chunks (of 128) per dst tile
CAP = CHUNKS * 128  # 1152 edge slots per tile
KCH = IN_DIM // 128  # 6 contraction chunks

TRACE = False
LAST_EXEC_NS = {}

_cache = {}


def _build_launch_a():
    # qkv/skip computed directly from x with host-fused weights:
    #   W_eff = W_emb @ [Wq|Wk|Wv|Wskip],  b_eff = b_emb @ [..] + [bq|bk|bv|bskip]
    nc = bacc.Bacc("TRN2", debug=False, num_devices=NCORES)
    xT = nc.dram_tensor("xT", [KCH * 128, NPAD], BF16, kind="ExternalInput").ap()
    wqkvs = nc.dram_tensor("wqkvs", [KCH * 128, 1600], BF16, kind="ExternalInput").ap()
    bqkvs = nc.dram_tensor("bqkvs", [128, 1600], BF16, kind="ExternalInput").ap()
    qkv_out = nc.dram_tensor("qkv_out", [NPAD, 1536], BF16, kind="ExternalOutput").ap()
    skip_out = nc.dram_tensor("skip_out", [NPAD, OUT_DIM], F32, kind="ExternalOutput").ap()

    with tile.TileContext(nc) as tc:
        with (
            tc.tile_pool(name="const", bufs=1) as cpool,
            tc.tile_pool(name="work", bufs=3) as wpool,
            tc.tile_pool(name="psum_qkv", bufs=2, space="PSUM") as pq,
        ):
            xT_sb = cpool.tile([128, KCH * NPAD], BF16)
            wqkvs_sb = cpool.tile([128, KCH * 1600], BF16)
            bqkvs_sb = cpool.tile([128, 1600], BF16)
            for k in range(KCH):
                nc.sync.dma_start(xT_sb[:, k * NPAD:(k + 1) * NPAD], xT[ts(k, 128), :])
                nc.sync.dma_start(wqkvs_sb[:, k * 1600:(k + 1) * 1600], wqkvs[ts(k, 128), :])
            nc.sync.dma_start(bqkvs_sb[:], bqkvs[:])

            for m in range(TILES):
                qkvs_ps = pq.tile([128, 1600], F32, tag="qkvs")
                for k in range(KCH):
                    for n0, nw in ((0, 512), (512, 512), (1024, 512), (1536, 64)):
                        nc.tensor.matmul(
                            qkvs_ps[:, n0:n0 + nw],
                            lhsT=xT_sb[:, k * NPAD + m * 128: k * NPAD + (m + 1) * 128],
                            rhs=wqkvs_sb[:, k * 1600 + n0: k * 1600 + n0 + nw],
                            start=(k == 0),
                            stop=(k == KCH - 1),
                        )
                qkv_sb = wpool.tile([128, 1536], BF16, tag="qkv")
                nc.vector.tensor_add(qkv_sb[:], qkvs_ps[:, :1536], bqkvs_sb[:, :1536])
                skip_sb = wpool.tile([128, OUT_DIM], F32, tag="skip")
                nc.vector.tensor_add(skip_sb[:], qkvs_ps[:, 1536:1600], bqkvs_sb[:, 1536:1600])
                nc.sync.dma_start(qkv_out[ts(m, 128), :], qkv_sb[:])
                nc.sync.dma_start(skip_out[ts(m, 128), :], skip_sb[:])
    nc.compile()
    return nc


def _build_launch_b():
    nc = bacc.Bacc("TRN2", debug=False, num_devices=NCORES)
    qg = nc.dram_tensor("qg", [TILES, 128, CHUNKS * HC], BF16, kind="ExternalInput").ap()
    kg = nc.dram_tensor("kg", [TILES, 128, CHUNKS * HC], BF16, kind="ExternalInput").ap()
    vg = nc.dram_tensor("vg", [TILES, 128, CHUNKS * HC], BF16, kind="ExternalInput").ap()
    ind = nc.dram_tensor("ind", [TILES, 128, CHUNKS * 128], BF16, kind="ExternalInput").ap()
    skip = nc.dram_tensor("skip", [TILES, 128, OUT_DIM], F32, kind="ExternalInput").ap()
    indng = nc.dram_tensor("indng", [TILES, 128, B], BF16, kind="ExternalInput").ap()
    pooled = nc.dram_tensor("pooled", [B, OUT_DIM], F32, kind="ExternalOutput").ap()

    scale = 1.0 / np.sqrt(np.float32(OUT_DIM))

    with tile.TileContext(nc) as tc:
        with (
            tc.tile_pool(name="io", bufs=3) as iop,
            tc.tile_pool(name="work", bufs=4) as wp,
            tc.tile_pool(name="psA", bufs=2, space="PSUM") as psA,
            tc.tile_pool(name="psB", bufs=2, space="PSUM") as psB,
            tc.tile_pool(name="psP", bufs=1, space="PSUM") as psP,
            tc.tile_pool(name="outp", bufs=1) as outp,
        ):
            pool_ps = psP.tile([B, OUT_DIM], F32)
            for t in range(TILES):
                qg_sb = iop.tile([128, CHUNKS * HC], BF16, tag="qg")
                kg_sb = iop.tile([128, CHUNKS * HC], BF16, tag="kg")
                vg_sb = iop.tile([128, CHUNKS * HC], BF16, tag="vg")
                ind_sb = iop.tile([128, CHUNKS * 128], BF16, tag="ind")
                skip_sb = iop.tile([128, OUT_DIM], F32, tag="skip")
                indng_sb = iop.tile([128, B], BF16, tag="indng")
                nc.sync.dma_start(qg_sb[:], qg[t])
                nc.sync.dma_start(kg_sb[:], kg[t])
                nc.sync.dma_start(vg_sb[:], vg[t])
                nc.sync.dma_start(ind_sb[:], ind[t])
                nc.sync.dma_start(skip_sb[:], skip[t])
                nc.sync.dma_start(indng_sb[:], indng[t])

                num_ps = psA.tile([128, HC], F32, tag="num")
                den_ps = psB.tile([128, HEADS], F32, tag="den")
                # process chunks in pairs: one DVE/ACT instruction covers 2 chunks
                for c0 in range(0, CHUNKS, 2):
                    w2 = min(2, CHUNKS - c0)
                    wd = w2 * HC
                    qk = wp.tile([128, 2 * HC], BF16, tag="qk")
                    nc.vector.tensor_mul(
                        qk[:, :wd], qg_sb[:, c0 * HC:(c0 + w2) * HC],
                        kg_sb[:, c0 * HC:(c0 + w2) * HC])
                    # fold halves at DVE 2x, then half-size reduce at 1x
                    qkh = wp.tile([128, HC], BF16, tag="qkh")
                    h3 = qk[:, :wd].rearrange("p (h c) -> p h c", h=w2 * HEADS)
                    nc.vector.tensor_add(
                        qkh[:, :wd // 2].rearrange("p (h c) -> p h c", h=w2 * HEADS),
                        h3[:, :, 0:OUT_DIM // 2], h3[:, :, OUT_DIM // 2:OUT_DIM])
                    s_f = wp.tile([128, 2 * HEADS], F32, tag="s")
                    nc.vector.reduce_sum(
                        s_f[:, :w2 * HEADS],
                        qkh[:, :wd // 2].rearrange("p (h c) -> p h c", h=w2 * HEADS),
                        axis=mybir.AxisListType.X,
                    )
                    w_bf = wp.tile([128, 2 * HC], BF16, tag="w")
                    nc.scalar.activation(
                        out=w_bf[:, :wd].rearrange("p (h c) -> p h c", h=w2 * HEADS),
                        in_=s_f[:, :w2 * HEADS].rearrange("p h -> p h ()").to_broadcast(
                            [128, w2 * HEADS, OUT_DIM]),
                        func=mybir.ActivationFunctionType.Exp,
                        scale=float(scale),
                    )
                    wv = wp.tile([128, 2 * HC], BF16, tag="wv")
                    nc.vector.tensor_mul(wv[:, :wd], vg_sb[:, c0 * HC:(c0 + w2) * HC], w_bf[:, :wd])
                    for c in range(c0, c0 + w2):
                        j = c - c0
                        nc.tensor.matmul(
                            num_ps[:], lhsT=ind_sb[:, ts(c, 128)], rhs=wv[:, ts(j, HC)],
                            start=(c == 0), stop=(c == CHUNKS - 1),
                        )
                        nc.tensor.matmul(
                            den_ps[:], lhsT=ind_sb[:, ts(c, 128)],
                            rhs=w_bf[:, ts(j, HC)].rearrange("p (h c) -> p h c", h=HEADS)[:, :, 0],
                            start=(c == 0), stop=(c == CHUNKS - 1),
                        )
                # epilogue: out = mean_h(num/den) + skip
                rec = wp.tile([128, HEADS], F32, tag="rec")
                nc.vector.tensor_scalar(
                    out=rec[:], in0=den_ps[:],
                    scalar1=float(HEADS), scalar2=1e-12,
                    op0=mybir.AluOpType.mult, op1=mybir.AluOpType.add,
                )
                nc.vector.reciprocal(rec[:], rec[:])
                mh = wp.tile([128, HC], BF16, tag="mh")
                nc.vector.tensor_mul(
                    mh[:].rearrange("p (h c) -> p h c", h=HEADS),
                    num_ps[:].rearrange("p (h c) -> p h c", h=HEADS),
                    rec[:].rearrange("p h -> p h ()").to_broadcast([128, HEADS, OUT_DIM]),
                )
                out_sb = wp.tile([128, OUT_DIM], BF16, tag="out")
                msum = wp.tile([128, OUT_DIM], F32, tag="msum")
                nc.vector.reduce_sum(
                    msum[:],
                    mh[:].rearrange("p (h c) -> p c h", h=HEADS),
                    axis=mybir.AxisListType.X,
                )
                nc.vector.tensor_add(out_sb[:], msum[:], skip_sb[:])
                nc.tensor.matmul(
                    pool_ps[:], lhsT=indng_sb[:], rhs=out_sb[:],
                    start=(t == 0), stop=(t == TILES - 1),
                )
            pooled_sb = outp.tile([B, OUT_DIM], F32)
            nc.vector.tensor_copy(pooled_sb[:], pool_ps[:])
            nc.sync.dma_start(pooled[:], pooled_sb[:])
    nc.compile()
    return nc


def _get_programs():
    if "A" not in _cache:
        _cache["A"] = _build_launch_a()
    if "B" not in _cache:
        _cache["B"] = _build_launch_b()
    return _cache["A"], _cache["B"]


LAST_TRACE_PATH = {}


def _ensure_hook_shim():
    import sys
    import types

    if "antenv.axon_hooks" in sys.modules:
        return
    mod = types.ModuleType("antenv.axon_hooks")
    holder = [None]
    mod.set_axon_ntff_profile_hook = lambda h: holder.__setitem__(0, h)
    mod.get_axon_ntff_profile_hook = lambda: holder[0]
    sys.modules["antenv.axon_hooks"] = mod
    import antenv

    antenv.axon_hooks = mod
    from trn_agent_boot.trn_boot import _ntff_profile_via_ctypes

    mod.set_axon_ntff_profile_hook(
        _ntff_profile_via_ctypes("/opt/axon/libaxon_pjrt.so")
    )


def _run(nc, in_maps, label):
    if not TRACE:
        res = bass_utils.run_bass_kernel_spmd(nc, in_maps, list(range(NCORES)))
        return res.results

    import glob
    import os
    import tempfile

    from concourse import bass2jax
    from concourse._compat import FishPath
    import gauge.profiler

    _ensure_hook_shim()
    import antenv.axon_hooks as hooks

    tmpdir = tempfile.mkdtemp(prefix=f"bass_{label}_")
    with hooks.get_axon_ntff_profile_hook()(tmpdir, [0]):
        results = bass2jax.run_bass_via_pjrt(nc, in_maps, n_cores=NCORES)
    exec_ns = None
    try:
        ntffs = glob.glob(os.path.join(tmpdir, "*_body*.ntff"))
        if ntffs:
            profile = gauge.profiler.Profile(
                profile_path=FishPath(tmpdir),
                kernel_dev_mode=True,
                profile_on_exit=False,
                bass_kernel=nc.m,
                offline_processing=True,
                fname="*_body*",
            )
            prs = profile.to_perfetto(model_index=(0,))
            if prs:
                exec_ns = max(p.exec_time_ns for p in prs)
                LAST_TRACE_PATH[label] = (tmpdir, [p.trace_path for p in prs])
        else:
            print(f"[{label}] no ntff files in {tmpdir}: {os.listdir(tmpdir)}")
    except Exception as e:  # profiling must never break the run
        print(f"[{label}] profile processing failed: {type(e).__name__}: {e}")
    LAST_EXEC_NS[label] = exec_ns
    return results


def kernel(x, edge_index, batch, W_emb, b_emb, Wq, bq, Wk, bk, Wv, bv, Wskip, bskip):
    x = np.asarray(x, np.float32)
    edge_index = np.asarray(edge_index)
    batch_np = np.asarray(batch)
    ncA, ncB = _get_programs()

    # ---- host prep for launch A: fold W_emb/b_emb into the qkv/skip weights ----
    wcat = np.concatenate(
        [np.asarray(Wq, np.float32), np.asarray(Wk, np.float32),
         np.asarray(Wv, np.float32), np.asarray(Wskip, np.float32)], axis=1
    )  # [768, 1600]
    bcat = np.concatenate(
        [np.asarray(bq, np.float32), np.asarray(bk, np.float32),
         np.asarray(bv, np.float32), np.asarray(bskip, np.float32)]
    )  # [1600]
    wemb_f = np.asarray(W_emb, np.float32)
    bemb_f = np.asarray(b_emb, np.float32)
    wqkvs = (wemb_f @ wcat).astype(NP_BF16)          # [768, 1600]
    bqkvs = (bemb_f @ wcat + bcat).astype(np.float32)
    bqkvs_rep = np.broadcast_to(bqkvs.astype(NP_BF16), (128, 1600)).copy()

    xpad = np.zeros((NCORES * NPAD, IN_DIM), NP_BF16)
    for c in range(NCORES):
        xpad[c * NPAD: c * NPAD + NPC] = x[c * NPC:(c + 1) * NPC].astype(NP_BF16)
    in_maps_a = []
    for c in range(NCORES):
        xT = np.ascontiguousarray(xpad[c * NPAD:(c + 1) * NPAD].T)  # [768, 6272]
        in_maps_a.append({"xT": xT, "wqkvs": wqkvs, "bqkvs": bqkvs_rep})
    res_a = _run(ncA, in_maps_a, "A")

    # ---- host mid: assemble Q,K,V and build edge-sorted gathers ----
    Q = np.concatenate([res_a[c]["qkv_out"][:NPC, 0:512] for c in range(NCORES)])
    K = np.concatenate([res_a[c]["qkv_out"][:NPC, 512:1024] for c in range(NCORES)])
    V = np.concatenate([res_a[c]["qkv_out"][:NPC, 1024:1536] for c in range(NCORES)])

    src = np.asarray(edge_index[0], np.int64)
    dst = np.asarray(edge_index[1], np.int64)
    core = dst // NPC
    local = dst - core * NPC
    tile_g = core * TILES + local // 128  # 0 .. 8*49-1
    dloc = local % 128
    order = np.argsort(tile_g, kind="stable")
    tg_s, src_s, dloc_s = tile_g[order], src[order], dloc[order]
    ntile = NCORES * TILES
    counts = np.bincount(tg_s, minlength=ntile)
    if counts.max() > CAP:
        raise RuntimeError(f"tile capacity exceeded: {counts.max()} > {CAP}")
    starts = np.zeros(ntile, np.int64)
    starts[1:] = np.cumsum(counts)[:-1]
    pos = np.arange(E) - starts[tg_s]
    rows = tg_s * CAP + pos  # slot in [ntile*CAP]

    src_pad = np.zeros(ntile * CAP, np.int64)
    src_pad[rows] = src_s
    dst_pad = np.full(ntile * CAP, -1, np.int64)
    dst_pad[rows] = dloc_s
    dstg_pad = np.zeros(ntile * CAP, np.int64)
    dstg_pad[rows] = dst[order]

    def tileize(a):  # [ntile*CAP, D] -> [ntile, 128, CHUNKS*D]
        d = a.shape[1]
        return np.ascontiguousarray(
            a.reshape(ntile, CHUNKS, 128, d).transpose(0, 2, 1, 3).reshape(ntile, 128, CHUNKS * d)
        )

    qg_t = tileize(Q[dstg_pad])
    kg_t = tileize(K[src_pad])
    vg_t = tileize(V[src_pad])
    ind_t = tileize(
        (dst_pad[:, None] == np.arange(128)[None, :]).astype(NP_BF16)
    )

    batch_pad = np.full(NCORES * NPAD, -1, np.int64)
    for c in range(NCORES):
        batch_pad[c * NPAD: c * NPAD + NPC] = batch_np[c * NPC:(c + 1) * NPC]
    indng = (batch_pad[:, None] == np.arange(B)[None, :]).astype(NP_BF16)
    indng = indng.reshape(NCORES, TILES, 128, B)

    in_maps_b = []
    for c in range(NCORES):
        sl = slice(c * TILES, (c + 1) * TILES)
        in_maps_b.append({
            "qg": qg_t[sl], "kg": kg_t[sl], "vg": vg_t[sl], "ind": ind_t[sl],
            "skip": np.ascontiguousarray(
                res_a[c]["skip_out"].reshape(TILES, 128, OUT_DIM)),
            "indng": indng[c],
        })
    res_b = _run(ncB, in_maps_b, "B")

    pooled = np.zeros((B, OUT_DIM), np.float64)
    for c in range(NCORES):
        pooled += res_b[c]["pooled"].astype(np.float64)
    cnt = np.bincount(batch_np, minlength=B).astype(np.float64)
    pooled /= np.maximum(cnt, 1.0)[:, None]
    return pooled.astype(np.float32)



# revision 3
# speedup vs baseline: 1.0550x; 1.0550x over previous
"""GraphTransformer (TransformerConv + mean-pool) on 8 trn2 NeuronCores — v2.

Two launches, nodes sharded 8 ways (dst-contiguous blocks):

Launch A (per core, 6250 nodes padded to 6272 = 49 tiles of 128):
    qkv = x @ W_eff (fp8 DoubleRow matmuls, W_eff = W_emb @ [Wq|Wk|Wv] x32)
    skip = x @ (W_emb @ Wskip)  (bf16 matmuls)
    Biases folded out: bq_eff host-added to Q before gather; bk_eff cancels
    in softmax; bv_eff and skip bias applied in the host epilogue.

Host mid: sort edges by dst tile; gather per-edge q'/k in transposed
    head-pair layout qgT/kgT [128=(2h x 64c), 4g, 1152e] fp8, v in straight
    layout vg [128e, 9ch x 512hc] fp8; one-hot dst indicator ind bf16.

Launch B (per core, 49 dst tiles x 9 chunks of 128 edge slots):
    qkT = qgT * kgT                  (one DVE TT at 2x per tile)
    s[e,2h] = qkT_g-slice^T @ mask   (4 tiny PE matmuls per chunk)
    w2 = Exp(s * scale) expanded x2  (ACT, psum->sbuf)
    wv = vg * w2-broadcast           (one DVE TT at 2x, stride-0 mid dim)
    num += ind^T @ wv; den += ind^T @ w2[...,0]   (PE per chunk)
    out: num,den (bf16) per tile -> host

Host: msg = num/den + bv_eff (where den>0), out = mean_h(msg) + skip + bsk,
    pooled segment-mean by batch id.
"""

import numpy as np
import ml_dtypes

import concourse.bass as bass
from concourse import bacc
import concourse.mybir as mybir
import concourse.tile as tile
from concourse import bass_utils
from concourse.bass import ts

BF16 = mybir.dt.bfloat16
F32 = mybir.dt.float32
FP8 = mybir.dt.float8e4
NP_BF16 = ml_dtypes.bfloat16
NP_FP8 = ml_dtypes.float8_e4m3

N, E, B = 50000, 400000, 64
IN_DIM, OUT_DIM, HEADS = 768, 64, 8
HC = HEADS * OUT_DIM  # 512
NCORES = 8
NPC = N // NCORES  # 6250
TILES = 49
NPAD = TILES * 128  # 6272
CHUNKS = 9
CAP = CHUNKS * 128  # 1152
NT = NCORES * TILES  # 392 tiles globally
WSCALE = 32.0
SCALE = 1.0 / np.sqrt(np.float32(OUT_DIM))

CAST_WIRE = False  # fp8 on the wire + SWDGE cast-DMA to bf16

TRACE = False
LAST_EXEC_NS = {}
LAST_TRACE_PATH = {}

_cache = {}


def _build_launch_a():
    nc = bacc.Bacc("TRN2", debug=False, num_devices=NCORES)
    xT8 = nc.dram_tensor("xT8", [IN_DIM, NPAD], FP8, kind="ExternalInput").ap()
    xT16 = nc.dram_tensor("xT16", [IN_DIM, NPAD], BF16, kind="ExternalInput").ap()
    w8 = nc.dram_tensor("w8", [IN_DIM, HC * 3], FP8, kind="ExternalInput").ap()
    wsk = nc.dram_tensor("wsk", [IN_DIM, OUT_DIM], BF16, kind="ExternalInput").ap()
    qkv8 = nc.dram_tensor("qkv8", [NPAD, HC * 3], FP8, kind="ExternalOutput").ap()
    skip16 = nc.dram_tensor("skip16", [NPAD, OUT_DIM], BF16, kind="ExternalOutput").ap()

    with tile.TileContext(nc) as tc, nc.allow_low_precision(reason="fp8 kernel"):
        with (
            tc.tile_pool(name="const", bufs=1) as cp,
            tc.tile_pool(name="work", bufs=3) as wp,
            tc.tile_pool(name="ps", bufs=2, space="PSUM") as pp,
        ):
            x8_sb = cp.tile([128, 3, 2, NPAD], FP8)
            x16_sb = cp.tile([128, 6, NPAD], BF16)
            w8_sb = cp.tile([128, 3, 2, HC * 3], FP8)
            wsk_sb = cp.tile([128, 6, OUT_DIM], BF16)
            for kp in range(3):
                for j in range(2):
                    k = kp * 2 + j
                    nc.sync.dma_start(x8_sb[:, kp, j, :], xT8[ts(k, 128), :])
                    nc.sync.dma_start(w8_sb[:, kp, j, :], w8[ts(k, 128), :])
                    nc.sync.dma_start(x16_sb[:, k, :], xT16[ts(k, 128), :])
                    nc.sync.dma_start(wsk_sb[:, k, :], wsk[ts(k, 128), :])

            for m in range(TILES):
                ps = pp.tile([128, HC * 3 + OUT_DIM], F32, tag="ps")
                # 2KB psum zero-regions: each 512-col region holds two 256-col
                # DR output slices -> region-level start/stop with kp-inner
                # order inside each n-pair.
                for r in range(3):
                    for i, (n, kp) in enumerate(
                            [(2 * r, 0), (2 * r, 1), (2 * r, 2),
                             (2 * r + 1, 0), (2 * r + 1, 1), (2 * r + 1, 2)]):
                        nc.tensor.matmul(
                            ps[:, ts(n, 256)],
                            lhsT=x8_sb[:, kp, :, ts(m, 128)],
                            rhs=w8_sb[:, kp, :, ts(n, 256)],
                            start=(i == 0), stop=(i == 5),
                            perf_mode=mybir.MatmulPerfMode.DoubleRow,
                        )
                for k in range(6):
                    nc.tensor.matmul(
                        ps[:, HC * 3:],
                        lhsT=x16_sb[:, k, ts(m, 128)],
                        rhs=wsk_sb[:, k, :],
                        start=(k == 0), stop=(k == 5),
                    )
                qkv_sb = wp.tile([128, HC * 3], FP8, tag="qkv")
                # split the psum->sbuf copy between ACT and DVE
                nc.scalar.activation(
                    out=qkv_sb[:, :1024], in_=ps[:, :1024],
                    func=mybir.ActivationFunctionType.Copy, scale=1.0 / WSCALE)
                nc.vector.tensor_scalar(
                    out=qkv_sb[:, 1024:], in0=ps[:, 1024:HC * 3],
                    scalar1=1.0 / WSCALE, scalar2=None,
                    op0=mybir.AluOpType.mult)
                skip_sb = wp.tile([128, OUT_DIM], BF16, tag="skip")
                nc.vector.tensor_copy(skip_sb[:], ps[:, HC * 3:])
                nc.sync.dma_start(qkv8[ts(m, 128), :], qkv_sb[:])
                nc.sync.dma_start(skip16[ts(m, 128), :], skip_sb[:])
    nc.compile()
    return nc


def _build_launch_b():
    nc = bacc.Bacc("TRN2", debug=False, num_devices=NCORES)
    wire = FP8 if CAST_WIRE else BF16
    qgT = nc.dram_tensor("qgT", [TILES, 128, 4 * CAP], wire, kind="ExternalInput").ap()
    kgT = nc.dram_tensor("kgT", [TILES, 128, 4 * CAP], wire, kind="ExternalInput").ap()
    vg = nc.dram_tensor("vg", [TILES, 128, CHUNKS * HC], wire, kind="ExternalInput").ap()
    ind = nc.dram_tensor("ind", [TILES, 128, CHUNKS * 128], BF16, kind="ExternalInput").ap()
    nd = nc.dram_tensor("nd", [TILES, 128, HC + HEADS], BF16, kind="ExternalOutput").ap()

    with tile.TileContext(nc) as tc, nc.allow_low_precision(reason="fp8 kernel"):
        with (
            tc.tile_pool(name="const", bufs=1) as cp,
            tc.tile_pool(name="io", bufs=2) as iop,
            tc.tile_pool(name="work", bufs=2) as wp,
            tc.tile_pool(name="psn", bufs=2, space="PSUM") as psn,
            tc.tile_pool(name="psd", bufs=2, space="PSUM") as psd,
            tc.tile_pool(name="pss", bufs=2, space="PSUM") as pss,
        ):
            mask_sb = cp.tile([128, 2], BF16)
            nc.vector.memset(mask_sb[:], 0.0)
            nc.vector.memset(mask_sb[0:64, 0:1], 1.0)
            nc.vector.memset(mask_sb[64:128, 1:2], 1.0)

            for t in range(TILES):
                qgT_sb = iop.tile([128, 4 * CAP], BF16, tag="qgT")
                kgT_sb = iop.tile([128, 4 * CAP], BF16, tag="kgT")
                vg_sb = iop.tile([128, CHUNKS * HC], BF16, tag="vg")
                ind_sb = iop.tile([128, CHUNKS * 128], BF16, tag="ind")
                dma = nc.gpsimd.dma_start if CAST_WIRE else nc.sync.dma_start
                dma(qgT_sb[:], qgT[t])
                dma(kgT_sb[:], kgT[t])
                dma(vg_sb[:], vg[t])
                nc.sync.dma_start(ind_sb[:], ind[t])

                qkT_sb = wp.tile([128, 4 * CAP], BF16, tag="qkT")
                nc.vector.tensor_mul(qkT_sb[:], qgT_sb[:], kgT_sb[:])
                qkT_g = qkT_sb.rearrange("p (g e) -> p g e", g=4)

                w64_sb = wp.tile([128, CHUNKS * HC], BF16, tag="w64")
                num_ps = psn.tile([128, HC], F32, tag="num")
                den_ps = psd.tile([128, 512], F32, tag="den")

                # scores for all chunks into one psum bank: s_all[:, c*8+h]
                s_ps = pss.tile([128, 512], F32, tag="s")
                for c in range(CHUNKS):
                    for g in range(4):
                        nc.tensor.matmul(
                            s_ps[:, c * 8 + 2 * g: c * 8 + 2 * g + 2],
                            lhsT=qkT_g[:, g, ts(c, 128)],
                            rhs=mask_sb[:],
                            start=(c == 0 and g == 0),
                            stop=(c == CHUNKS - 1 and g == 3),
                        )
                # one exp+expand for the whole tile: w64[(c h), j] = exp(s*scale)
                nc.scalar.activation(
                    out=w64_sb.rearrange("p (f j) -> p f j", f=CHUNKS * HEADS),
                    in_=s_ps[:, :CHUNKS * HEADS].rearrange(
                        "p f -> p f ()").to_broadcast(
                        [128, CHUNKS * HEADS, OUT_DIM]),
                    func=mybir.ActivationFunctionType.Exp,
                    scale=float(SCALE),
                )
                for c in range(CHUNKS):
                    nc.tensor.matmul(
                        den_ps[:, :HEADS],
                        lhsT=ind_sb[:, ts(c, 128)],
                        rhs=w64_sb[:, ts(c, HC)].rearrange(
                            "p (h j) -> p h j", h=HEADS)[:, :, 0],
                        start=(c == 0), stop=(c == CHUNKS - 1),
                    )

                wv_sb = wp.tile([128, CHUNKS * HC], BF16, tag="wv")
                nc.vector.tensor_mul(wv_sb[:], vg_sb[:], w64_sb[:])
                for c in range(CHUNKS):
                    nc.tensor.matmul(
                        num_ps[:],
                        lhsT=ind_sb[:, ts(c, 128)],
                        rhs=wv_sb[:, ts(c, HC)],
                        start=(c == 0), stop=(c == CHUNKS - 1),
                    )
                nd_sb = wp.tile([128, HC + HEADS], BF16, tag="nd")
                nc.scalar.activation(
                    out=nd_sb[:, :HC], in_=num_ps[:],
                    func=mybir.ActivationFunctionType.Copy)
                nc.vector.tensor_copy(nd_sb[:, HC:], den_ps[:, :HEADS])
                nc.sync.dma_start(nd[t], nd_sb[:])
    nc.compile()
    return nc


def _get_programs():
    if "A" not in _cache:
        _cache["A"] = _build_launch_a()
    if "B" not in _cache:
        _cache["B"] = _build_launch_b()
    return _cache["A"], _cache["B"]


def _ensure_hook_shim():
    import sys
    import types

    if "antenv.axon_hooks" in sys.modules:
        return
    mod = types.ModuleType("antenv.axon_hooks")
    holder = [None]
    mod.set_axon_ntff_profile_hook = lambda h: holder.__setitem__(0, h)
    mod.get_axon_ntff_profile_hook = lambda: holder[0]
    sys.modules["antenv.axon_hooks"] = mod
    import antenv

    antenv.axon_hooks = mod
    from trn_agent_boot.trn_boot import _ntff_profile_via_ctypes

    mod.set_axon_ntff_profile_hook(
        _ntff_profile_via_ctypes("/opt/axon/libaxon_pjrt.so")
    )


def _run(nc, in_maps, label):
    if not TRACE:
        res = bass_utils.run_bass_kernel_spmd(nc, in_maps, list(range(len(in_maps))))
        return res.results

    import glob
    import os
    import tempfile

    from concourse import bass2jax
    from concourse._compat import FishPath
    import gauge.profiler

    _ensure_hook_shim()
    import antenv.axon_hooks as hooks

    tmpdir = tempfile.mkdtemp(prefix=f"bass_{label}_")
    with hooks.get_axon_ntff_profile_hook()(tmpdir, [0]):
        results = bass2jax.run_bass_via_pjrt(nc, in_maps, n_cores=len(in_maps))
    exec_ns = None
    try:
        ntffs = glob.glob(os.path.join(tmpdir, "*_body*.ntff"))
        if ntffs:
            profile = gauge.profiler.Profile(
                profile_path=FishPath(tmpdir),
                kernel_dev_mode=True,
                profile_on_exit=False,
                bass_kernel=nc.m,
                offline_processing=True,
                fname="*_body*",
            )
            prs = profile.to_perfetto(model_index=(0,))
            if prs:
                exec_ns = max(p.exec_time_ns for p in prs)
                LAST_TRACE_PATH[label] = (tmpdir, [p.trace_path for p in prs])
        else:
            print(f"[{label}] no ntff files in {tmpdir}: {os.listdir(tmpdir)}")
    except Exception as e:  # profiling must never break the run
        print(f"[{label}] profile processing failed: {type(e).__name__}: {e}")
    LAST_EXEC_NS[label] = exec_ns
    return results


def kernel(x, edge_index, batch, W_emb, b_emb, Wq, bq, Wk, bk, Wv, bv, Wskip, bskip):
    x = np.asarray(x, np.float32)
    edge_index = np.asarray(edge_index)
    batch_np = np.asarray(batch)
    ncA, ncB = _get_programs()

    # ---- host prep for launch A ----
    wemb_f = np.asarray(W_emb, np.float32)
    bemb_f = np.asarray(b_emb, np.float32)
    wq_f, wk_f, wv_f = (np.asarray(w, np.float32) for w in (Wq, Wk, Wv))
    wskip_f = np.asarray(Wskip, np.float32)
    wqkv = wemb_f @ np.concatenate([wq_f, wk_f, wv_f], axis=1)  # [768, 1536]
    wsk_eff = wemb_f @ wskip_f  # [768, 64]
    bq_eff = bemb_f @ wq_f + np.asarray(bq, np.float32)  # [512]
    bv_eff = bemb_f @ wv_f + np.asarray(bv, np.float32)  # [512]
    bsk_eff = bemb_f @ wskip_f + np.asarray(bskip, np.float32)  # [64]

    w8 = (wqkv * WSCALE).astype(NP_FP8)
    wsk16 = wsk_eff.astype(NP_BF16)

    xpad = np.zeros((NCORES * NPAD, IN_DIM), np.float32)
    for c in range(NCORES):
        xpad[c * NPAD: c * NPAD + NPC] = x[c * NPC:(c + 1) * NPC]
    in_maps_a = []
    for c in range(NCORES):
        xT = np.ascontiguousarray(xpad[c * NPAD:(c + 1) * NPAD].T)
        in_maps_a.append({
            "xT8": xT.astype(NP_FP8),
            "xT16": xT.astype(NP_BF16),
            "w8": w8, "wsk": wsk16,
        })
    res_a = _run(ncA, in_maps_a, "A")

    # ---- host mid: assemble Q',K,V; edge-sorted gathers ----
    qkv = np.concatenate([res_a[c]["qkv8"][:NPC] for c in range(NCORES)])
    qkv_f = qkv.astype(np.float32)
    Qp8 = (qkv_f[:, 0:512] + bq_eff[None, :]).astype(NP_FP8)
    K8 = qkv[:, 512:1024]  # already fp8
    V8 = qkv[:, 1024:1536]
    skip = np.concatenate(
        [res_a[c]["skip16"][:NPC] for c in range(NCORES)]).astype(np.float32)

    src = np.asarray(edge_index[0], np.int64)
    dst = np.asarray(edge_index[1], np.int64)
    core = dst // NPC
    local = dst - core * NPC
    tile_g = core * TILES + local // 128
    dloc = local % 128
    order = np.argsort(tile_g, kind="stable")
    tg_s, src_s, dloc_s = tile_g[order], src[order], dloc[order]
    counts = np.bincount(tg_s, minlength=NT)
    if counts.max() > CAP:
        raise RuntimeError(f"tile capacity exceeded: {counts.max()} > {CAP}")
    starts = np.zeros(NT, np.int64)
    starts[1:] = np.cumsum(counts)[:-1]
    pos = np.arange(E) - starts[tg_s]
    rows = tg_s * CAP + pos

    src_pad = np.zeros(NT * CAP, np.int64)
    src_pad[rows] = src_s
    dst_pad = np.full(NT * CAP, -1, np.int64)
    dst_pad[rows] = dloc_s
    dstg_pad = np.zeros(NT * CAP, np.int64)
    dstg_pad[rows] = dst[order]
    pad_rows = np.ones(NT * CAP, np.bool_)
    pad_rows[rows] = False

    def t_layout(a8):  # [NT*CAP, 512] fp8 -> [NT, 128(hl,c), 4g*CAP]
        a4 = a8.reshape(NT, CAP, 4, 2, 64)  # [t, e, g, hl, c]
        return np.ascontiguousarray(
            a4.transpose(0, 3, 4, 2, 1).reshape(NT, 128, 4 * CAP))

    qg8 = Qp8[dstg_pad]
    qg8[pad_rows] = 0
    kg8 = K8[src_pad]
    kg8[pad_rows] = 0
    qgT_t = t_layout(qg8)
    kgT_t = t_layout(kg8)

    def tileize(a):  # [NT*CAP, D] -> [NT, 128, CHUNKS*D]
        d = a.shape[1]
        return np.ascontiguousarray(
            a.reshape(NT, CHUNKS, 128, d).transpose(0, 2, 1, 3).reshape(
                NT, 128, CHUNKS * d))

    vg_t = tileize(V8[src_pad])
    ind_t = tileize(
        (dst_pad[:, None] == np.arange(128)[None, :]).astype(NP_BF16))

    wire_np = NP_FP8 if CAST_WIRE else NP_BF16
    in_maps_b = []
    for c in range(NCORES):
        sl = slice(c * TILES, (c + 1) * TILES)
        in_maps_b.append({
            "qgT": qgT_t[sl].astype(wire_np), "kgT": kgT_t[sl].astype(wire_np),
            "vg": vg_t[sl].astype(wire_np), "ind": ind_t[sl],
        })
    res_b = _run(ncB, in_maps_b, "B")

    # ---- host epilogue ----
    nd = np.concatenate([res_b[c]["nd"][None] for c in range(NCORES)])
    nd = nd.astype(np.float32)  # [NCORES, TILES, 128, 520]
    nd = nd.reshape(NCORES, NPAD, HC + HEADS)
    num = np.concatenate([nd[c, :NPC, :HC] for c in range(NCORES)])
    den = np.concatenate([nd[c, :NPC, HC:] for c in range(NCORES)])
    num = num.reshape(N, HEADS, OUT_DIM)
    msg = num / np.maximum(den, 1e-30)[:, :, None]
    msg = np.where(
        (den > 0)[:, :, None],
        msg + bv_eff.reshape(HEADS, OUT_DIM)[None], 0.0)
    out = msg.mean(axis=1) + skip + bsk_eff[None, :]

    cnt = np.bincount(batch_np, minlength=B).astype(np.float64)
    pooled = np.zeros((B, OUT_DIM), np.float64)
    np.add.at(pooled, batch_np, out.astype(np.float64))
    pooled /= np.maximum(cnt, 1.0)[:, None]
    return pooled.astype(np.float32)
